# revision 6
# baseline (speedup 1.0000x reference)
"""Trainium2 Bass kernel for the CCM retrieval problem.

Reference computation (shapes: bs=64, N=1024, D=2048, H=128, C=65):
    z_x   = softmax(cos(all_f, emb)/T/sqrt(N))            [bs, N]
    hf    = head(all_f); hz = head(emb)                   [bs, H], [N, H]
    h1    = relu(BN(hf[b] @ A + b1 + hz[n] @ B))          [bs, N, H]
    y_zx  = softmax((h1 @ mix_w2 + mix_b2) @ clf_w + clf_b)  [bs, N, C]
    p_x   = softmax(sum_n cos(all_f, all_f)/T/sqrt(bs))   [bs]
    out   = z_x @ einsum('b,bnc->nc', p_x, y_zx)          [bs, C]

Device strategy: shard the queue axis N across 8 cores (128 rows each);
bs stays replicated so sum_x is core-local. Host folds the BN affines into
weights, pre-multiplies mix_w2 @ clf_w (W2C) so the [bs,N,D] intermediate
never exists, pre-transposes the activations into D-on-partitions layout,
and precomputes row norms and p_x (tiny, O(bs^2 D)). Each core returns
[64, 66]: columns 0:65 are exp(z-score) @ sum_x partial numerators,
column 65 the z_x softmax denominator partial; the host sums partials
over cores and divides.
"""

import numpy as np
import ml_dtypes

import concourse.bass as bass
import concourse.tile as tile
from concourse import bacc, mybir
from concourse import bass_utils

F32 = mybir.dt.float32
BF16 = mybir.dt.bfloat16
AX = mybir.AxisListType
ALU = mybir.AluOpType
ACTF = mybir.ActivationFunctionType

T = 0.07
BN_EPS = 1e-5
BS, D, N, H, C = 64, 2048, 1024, 128, 65
NCORES = 8
NLOC = N // NCORES          # 128 queue rows per core
DCH = D // 128              # 16 contraction chunks
GRP = 7                     # logits per psum bank: 7*65=455 <= 512
UCH = 4                     # h1 batches: 4 chunks x 16 batch rows
UB = BS // UCH


def _build(with_c0: bool):
    nc = bacc.Bacc("TRN2", target_bir_lowering=False, debug=False)

    d_etf = nc.dram_tensor("etf", [D, 192], F32, kind="ExternalInput")
    d_w1h = nc.dram_tensor("w1h", [D, H], F32, kind="ExternalInput")
    d_b1h = nc.dram_tensor("b1h", [H, 1], F32, kind="ExternalInput")
    d_wh2 = nc.dram_tensor("wh2", [H, H], F32, kind="ExternalInput")
    d_am = nc.dram_tensor("am", [H, H], F32, kind="ExternalInput")
    d_bm = nc.dram_tensor("bm", [H, H], F32, kind="ExternalInput")
    d_ca = nc.dram_tensor("ca", [H, 1], F32, kind="ExternalInput")
    d_cb = nc.dram_tensor("cb", [H, 1], F32, kind="ExternalInput")
    d_w2c = nc.dram_tensor("w2c", [H, C], BF16, kind="ExternalInput")
    d_ner = nc.dram_tensor("ner", [NLOC, 1], F32, kind="ExternalInput")
    d_nfb = nc.dram_tensor("nfb", [NLOC, BS], F32, kind="ExternalInput")
    d_pxb = nc.dram_tensor("pxb", [NLOC, BS], F32, kind="ExternalInput")
    if with_c0:
        d_c0 = nc.dram_tensor("c0t", [1, C], BF16, kind="ExternalInput")
    d_out = nc.dram_tensor("out_nd", [BS, C + 1], F32, kind="ExternalOutput")

    etf_view = d_etf.ap().rearrange("(i p) c -> p i c", p=128)
    w1h_view = d_w1h.ap().rearrange("(i p) h -> p i h", p=128)

    with tile.TileContext(nc) as tc:
        with (
            tc.tile_pool(name="consts", bufs=1) as consts,
            tc.tile_pool(name="big", bufs=1) as bigp,
            tc.tile_pool(name="work", bufs=2) as work,
            tc.tile_pool(name="pbig", bufs=3, space="PSUM") as pbig,
            tc.tile_pool(name="phead", bufs=2, space="PSUM") as phead,
            tc.tile_pool(name="psmall", bufs=2, space="PSUM") as psmall,
        ):
            # warmup: force the exp table set load at t=0 (overlaps the DMAs)
            warm = consts.tile([1, 1], F32)
            nc.vector.memset(warm, 0.0)
            warm2 = consts.tile([1, 1], F32)
            nc.scalar.activation(warm2, warm, ACTF.Exp)

            # ---- inputs to SBUF (activations chunked for pipelining) ----
            etf = bigp.tile([128, DCH, 192], F32)
            w1h_sb = bigp.tile([128, DCH, H], F32)
            for q in range(4):
                sl = slice(4 * q, 4 * (q + 1))
                eng = nc.sync if q % 2 == 0 else nc.scalar
                eng.dma_start(out=etf[:, sl, :], in_=etf_view[:, sl, :])
                eng = nc.scalar if q % 2 == 0 else nc.sync
                eng.dma_start(out=w1h_sb[:, sl, :], in_=w1h_view[:, sl, :])
            wh2_sb = consts.tile([H, H], F32)
            nc.sync.dma_start(out=wh2_sb, in_=d_wh2.ap())
            am_sb = consts.tile([H, H], F32)
            nc.sync.dma_start(out=am_sb, in_=d_am.ap())
            bm_sb = consts.tile([H, H], F32)
            nc.sync.dma_start(out=bm_sb, in_=d_bm.ap())
            b1h_sb = consts.tile([H, 1], F32)
            nc.sync.dma_start(out=b1h_sb, in_=d_b1h.ap())
            ca_sb = consts.tile([H, 1], F32)
            nc.sync.dma_start(out=ca_sb, in_=d_ca.ap())
            cb_sb = consts.tile([H, 1], F32)
            nc.sync.dma_start(out=cb_sb, in_=d_cb.ap())
            w2c_sb = consts.tile([H, C], BF16)
            nc.sync.dma_start(out=w2c_sb, in_=d_w2c.ap())
            ner_col = consts.tile([NLOC, 1], F32)
            nc.scalar.dma_start(out=ner_col, in_=d_ner.ap())
            nf_b = consts.tile([NLOC, BS], F32)
            nc.scalar.dma_start(out=nf_b, in_=d_nfb.ap())
            px_b = consts.tile([NLOC, BS], F32)
            nc.scalar.dma_start(out=px_b, in_=d_pxb.ap())
            if with_c0:
                ones_row_bf = consts.tile([1, 128], BF16)
                nc.vector.memset(ones_row_bf, 1.0)
                c0_sb = consts.tile([1, C], BF16)
                nc.sync.dma_start(out=c0_sb, in_=d_c0.ap())

            # ---- head: X1 = relu(W1h.T @ [embT | all_fT] + b1h) ----
            xt = phead.tile([128, 192], F32, tag="ph")
            for i in range(DCH):
                nc.tensor.matmul(
                    xt, w1h_sb[:, i, :], etf[:, i, :], start=(i == 0),
                    stop=(i == DCH - 1),
                )
            x1 = work.tile([128, 192], F32)
            nc.scalar.activation(x1, xt, ACTF.Relu, bias=b1h_sb)
            # layer 2 (no bias: head_b2 folded into ca/cb)
            x2p = phead.tile([128, 192], F32, tag="ph")
            nc.tensor.matmul(x2p, wh2_sb, x1)
            x2 = work.tile([128, 192], F32)
            nc.vector.tensor_copy(x2, x2p)
            # alphaT [H', bs], betaT [H', nloc] with constants folded in
            abp = phead.tile([128, 192], F32, tag="ph")
            nc.tensor.matmul(abp[:, 0:128], bm_sb, x2[:, 0:128])
            nc.tensor.matmul(abp[:, 128:192], am_sb, x2[:, 128:192])
            beta_bf = work.tile([128, NLOC], BF16)
            nc.vector.tensor_scalar_add(beta_bf, abp[:, 0:128], cb_sb)
            alpha_bf = work.tile([128, BS], BF16)
            nc.vector.tensor_scalar_add(alpha_bf, abp[:, 128:192], ca_sb)

            # ---- h1T batches: u_k[:, j, :] = relu(betaT + alphaT[:, 16k+j]) ----
            u_tiles = []
            for k in range(UCH):
                uk = bigp.tile([128, UB, NLOC], BF16, tag=f"u{k}")
                nc.vector.tensor_tensor(
                    uk,
                    beta_bf.unsqueeze(1).broadcast_to([128, UB, NLOC]),
                    alpha_bf[:, UB * k : UB * (k + 1)]
                    .unsqueeze(2)
                    .broadcast_to([128, UB, NLOC]),
                    op=ALU.add,
                )
                nc.vector.tensor_scalar_max(uk, uk, 0.0)
                u_tiles.append(uk)

            # ---- z_x scores (transposed): ezT[n, b] = exp(cos/(T*sqrt(N))) ----
            slp = psmall.tile([NLOC, BS], F32, tag="ps")
            for i in range(DCH):
                nc.tensor.matmul(
                    slp, etf[:, i, 0:128], etf[:, i, 128:192], start=(i == 0),
                    stop=(i == DCH - 1),
                )
            t3 = work.tile([NLOC, BS], F32)
            nc.vector.tensor_tensor(t3, slp, nf_b, op=ALU.mult)
            ez = work.tile([NLOC, BS], F32)
            nc.scalar.activation(ez, t3, ACTF.Exp, scale=ner_col)

            # ---- main loop: logits -> exp -> p_x/S weights -> weighted E ----
            e_t = bigp.tile([128, BS * C], F32)      # exp(logits), b-major
            e2_t = bigp.tile([128, C * BS], F32)     # weighted, c-major (b inner)
            s_t = work.tile([128, BS], F32)          # softmax denominators
            w_t = work.tile([128, BS], F32)          # p_x[b] / S[n, b]
            groups = [
                (g0, min(GRP, BS - g0 * GRP)) for g0 in range((BS + GRP - 1) // GRP)
            ]
            e2_v = e2_t.rearrange("p (c b) -> p c b", b=BS)
            for g0, nb in groups:
                pg = pbig.tile([128, GRP * C], F32, tag="pb")
                for j in range(nb):
                    b = g0 * GRP + j
                    h1t = u_tiles[b // UB][:, b % UB, :]
                    sl = pg[:, C * j : C * (j + 1)]
                    if with_c0:
                        nc.tensor.matmul(
                            sl, ones_row_bf, c0_sb, start=True, stop=False
                        )
                        nc.tensor.matmul(sl, h1t, w2c_sb, start=False, stop=True)
                    else:
                        nc.tensor.matmul(sl, h1t, w2c_sb, start=True, stop=True)
                gsl = slice(g0 * GRP, g0 * GRP + nb)
                egs = e_t[:, GRP * C * g0 : GRP * C * g0 + nb * C]
                nc.scalar.activation(egs, pg[:, 0 : nb * C], ACTF.Exp)
                e_v = egs.rearrange("p (b c) -> p b c", c=C)
                nc.vector.reduce_sum(s_t[:, gsl], e_v, axis=AX.X)
                nc.vector.reciprocal(w_t[:, gsl], s_t[:, gsl])
                nc.vector.tensor_tensor(
                    w_t[:, gsl], w_t[:, gsl], px_b[:, gsl], op=ALU.mult
                )
                e2_slice = e2_v[:, :, gsl].transpose([0, 2, 1])
                w_slice = w_t[:, gsl].unsqueeze(2).broadcast_to([128, nb, C])
                nc.gpsimd.tensor_tensor(e2_slice, e_v, w_slice, op=ALU.mult)

            # ---- sum over b, then partial out = ezT.T @ [sum_x | 1] ----
            sxo = work.tile([128, C + 1], F32)
            nc.vector.memset(sxo[:, C : C + 1], 1.0)
            nc.vector.reduce_sum(
                sxo[:, 0:C], e2_t.rearrange("p (c b) -> p c b", b=BS), axis=AX.X
            )
            onp = psmall.tile([BS, C + 1], F32, tag="ps")
            nc.tensor.matmul(onp, ez, sxo)
            on_s = work.tile([BS, C + 1], F32)
            nc.vector.tensor_copy(on_s, onp)
            nc.sync.dma_start(out=d_out.ap(), in_=on_s)

    nc.compile()
    return nc


_CACHE: dict = {}
LAST_RESULTS = None  # BassKernelResults of the most recent run (for profiling)


def _get_nc(with_c0: bool):
    if with_c0 not in _CACHE:
        _CACHE[with_c0] = _build(with_c0)
    return _CACHE[with_c0]


def kernel(
    all_f, embedding, all_y,
    head_w1, head_b1, head_g, head_beta, head_rm, head_rv, head_w2, head_b2,
    mix_w1, mix_b1, mix_g, mix_beta, mix_rm, mix_rv, mix_w2, mix_b2,
    clf_w, clf_b,
):
    f64 = np.float64
    sh = head_g.astype(f64) / np.sqrt(head_rv.astype(f64) + BN_EPS)
    th = head_beta.astype(f64) - head_rm.astype(f64) * sh
    w1h = (head_w1.astype(f64) * sh[None, :]).astype(np.float32)
    b1h = (head_b1.astype(f64) * sh + th).astype(np.float32)[:, None]
    sm = mix_g.astype(f64) / np.sqrt(mix_rv.astype(f64) + BN_EPS)
    tm = mix_beta.astype(f64) - mix_rm.astype(f64) * sm
    am = mix_w1[:H].astype(f64) * sm[None, :]
    bm = mix_w1[H:].astype(f64) * sm[None, :]
    cm = mix_b1.astype(f64) * sm + tm
    ca = (head_b2.astype(f64) @ am + cm).astype(np.float32)[:, None]
    cb = (head_b2.astype(f64) @ bm).astype(np.float32)[:, None]
    w2c = (mix_w2.astype(f64) @ clf_w.astype(f64)).astype(ml_dtypes.bfloat16)
    c0 = (mix_b2.astype(f64) @ clf_w.astype(f64) + clf_b.astype(f64)).astype(
        np.float32
    )
    with_c0 = bool(np.any(c0 != 0.0))

    af = np.ascontiguousarray(all_f, dtype=np.float32)
    emb = np.ascontiguousarray(embedding, dtype=np.float32)
    # input-side host prep: transposed layouts, row norms, p_x
    nf = 1.0 / np.sqrt((af.astype(f64) ** 2).sum(axis=1))          # [bs]
    nf_b = np.broadcast_to(nf[None, :], (NLOC, BS)).astype(np.float32)
    gscore = ((af @ af.T).astype(f64) * nf[:, None] * nf[None, :]).sum(axis=1)
    gscore = gscore / (T * np.sqrt(BS))
    pe = np.exp(gscore - gscore.max())
    px = pe / pe.sum()                                              # [bs]
    px_b = np.broadcast_to(px[None, :], (NLOC, BS)).astype(np.float32)
    aft = np.ascontiguousarray(af.T)                                # [D, bs]

    base = {
        "w1h": w1h,
        "b1h": b1h,
        "wh2": np.ascontiguousarray(head_w2, dtype=np.float32),
        "am": am.astype(np.float32),
        "bm": bm.astype(np.float32),
        "ca": ca,
        "cb": cb,
        "w2c": w2c,
        "nfb": np.ascontiguousarray(nf_b),
        "pxb": np.ascontiguousarray(px_b),
    }
    if with_c0:
        base["c0t"] = c0[None, :].astype(ml_dtypes.bfloat16)

    in_maps = []
    for i in range(NCORES):
        shard = emb[i * NLOC : (i + 1) * NLOC]
        etf = np.empty((D, 192), dtype=np.float32)
        etf[:, 0:128] = shard.T
        etf[:, 128:192] = aft
        ne = 1.0 / np.sqrt((shard.astype(f64) ** 2).sum(axis=1))
        ner = (ne / (T * np.sqrt(N))).astype(np.float32)[:, None]
        in_maps.append(dict(base, etf=etf, ner=ner))

    nc = _get_nc(with_c0)
    res = bass_utils.run_bass_kernel_spmd(nc, in_maps, core_ids=list(range(NCORES)))
    global LAST_RESULTS
    LAST_RESULTS = res
    parts = np.stack([r["out_nd"] for r in res.results], axis=0)  # [8, 64, 66]
    tot = parts.sum(axis=0)
    return (tot[:, :C] / tot[:, C : C + 1]).astype(np.float32)


# revision 9
# speedup vs baseline: 1.1562x; 1.1562x over previous
"""Trainium2 Bass kernel for the CCM retrieval problem.

Reference computation (shapes: bs=64, N=1024, D=2048, H=128, C=65):
    z_x   = softmax(cos(all_f, emb)/T/sqrt(N))            [bs, N]
    hf    = head(all_f); hz = head(emb)                   [bs, H], [N, H]
    h1    = relu(BN(hf[b] @ A + b1 + hz[n] @ B))          [bs, N, H]
    y_zx  = softmax((h1 @ mix_w2 + mix_b2) @ clf_w + clf_b)  [bs, N, C]
    p_x   = softmax(sum_n cos(all_f, all_f)/T/sqrt(bs))   [bs]
    out   = z_x @ einsum('b,bnc->nc', p_x, y_zx)          [bs, C]

Device strategy: shard the queue axis N across 8 cores (128 rows each);
bs stays replicated so sum_x is core-local. Host folds the BN affines into
weights, pre-multiplies mix_w2 @ clf_w (W2C) so the [bs,N,D] intermediate
never exists, pre-transposes the activations into D-on-partitions layout
(bf16), and precomputes row norms and p_x (tiny, O(bs^2 D)). Each core
returns [64, 66]: columns 0:65 are exp(z-score) @ sum_x partial numerators,
column 65 the z_x softmax denominator partial; the host sums partials over
cores and divides.
"""

import numpy as np
import ml_dtypes

import concourse.bass as bass
import concourse.tile as tile
from concourse import bacc, mybir
from concourse import bass_utils

F32 = mybir.dt.float32
BF16 = mybir.dt.bfloat16
AX = mybir.AxisListType
ALU = mybir.AluOpType
ACTF = mybir.ActivationFunctionType

T = 0.07
BN_EPS = 1e-5
BS, D, N, H, C = 64, 2048, 1024, 128, 65
NCORES = 8
NLOC = N // NCORES          # 128 queue rows per core
DCH = D // 128              # 16 contraction chunks
GRP = 8                     # logits per (2-bank) psum tile, 4 per bank
SG = BS // GRP              # 8 supergroups
UCH = 4                     # h1 quarters: 4 x 16 batch rows
UB = BS // UCH


def _build(with_c0: bool):
    nc = bacc.Bacc("TRN2", target_bir_lowering=False, debug=False)

    d_etf = nc.dram_tensor("etf", [D, 192], BF16, kind="ExternalInput")
    d_w1h = nc.dram_tensor("w1h", [D, H], BF16, kind="ExternalInput")
    d_b1h = nc.dram_tensor("b1h", [H, 1], F32, kind="ExternalInput")
    d_wh2 = nc.dram_tensor("wh2", [H, H], BF16, kind="ExternalInput")
    d_am = nc.dram_tensor("am", [H, H], BF16, kind="ExternalInput")
    d_bm = nc.dram_tensor("bm", [H, H], BF16, kind="ExternalInput")
    d_ca = nc.dram_tensor("ca", [H, 1], F32, kind="ExternalInput")
    d_cb = nc.dram_tensor("cb", [H, 1], F32, kind="ExternalInput")
    d_w2c = nc.dram_tensor("w2c", [H, C], BF16, kind="ExternalInput")
    d_ner = nc.dram_tensor("ner", [NLOC, 1], F32, kind="ExternalInput")
    d_nfb = nc.dram_tensor("nfb", [NLOC, BS], F32, kind="ExternalInput")
    d_pxb = nc.dram_tensor("pxb", [NLOC, BS], F32, kind="ExternalInput")
    if with_c0:
        d_c0 = nc.dram_tensor("c0t", [1, C], BF16, kind="ExternalInput")
    d_out = nc.dram_tensor("out_nd", [BS, C + 1], F32, kind="ExternalOutput")

    etf_view = d_etf.ap().rearrange("(i p) c -> p i c", p=128)
    w1h_view = d_w1h.ap().rearrange("(i p) h -> p i h", p=128)

    with tile.TileContext(nc) as tc:
        with (
            tc.tile_pool(name="consts", bufs=1) as consts,
            tc.tile_pool(name="big", bufs=1) as bigp,
            tc.tile_pool(name="work", bufs=2) as work,
            tc.tile_pool(name="pbig", bufs=2, space="PSUM") as pbig,
            tc.tile_pool(name="phead", bufs=2, space="PSUM") as phead,
            tc.tile_pool(name="psmall", bufs=2, space="PSUM") as psmall,
        ):
            # warmup: force the exp table set load at t=0 (overlaps the DMAs)
            warm = consts.tile([1, 1], F32)
            nc.vector.memset(warm, 0.0)
            warm2 = consts.tile([1, 1], F32)
            nc.scalar.activation(warm2, warm, ACTF.Exp)

            # ---- inputs to SBUF (activations chunked for pipelining) ----
            etf = bigp.tile([128, DCH, 192], BF16)
            w1h_sb = bigp.tile([128, DCH, H], BF16)
            for q in range(4):
                sl = slice(4 * q, 4 * (q + 1))
                eng = nc.sync if q % 2 == 0 else nc.scalar
                eng.dma_start(out=etf[:, sl, :], in_=etf_view[:, sl, :])
                eng = nc.scalar if q % 2 == 0 else nc.sync
                eng.dma_start(out=w1h_sb[:, sl, :], in_=w1h_view[:, sl, :])
            wh2_sb = consts.tile([H, H], BF16)
            nc.sync.dma_start(out=wh2_sb, in_=d_wh2.ap())
            am_sb = consts.tile([H, H], BF16)
            nc.sync.dma_start(out=am_sb, in_=d_am.ap())
            bm_sb = consts.tile([H, H], BF16)
            nc.sync.dma_start(out=bm_sb, in_=d_bm.ap())
            b1h_sb = consts.tile([H, 1], F32)
            nc.sync.dma_start(out=b1h_sb, in_=d_b1h.ap())
            ca_sb = consts.tile([H, 1], F32)
            nc.sync.dma_start(out=ca_sb, in_=d_ca.ap())
            cb_sb = consts.tile([H, 1], F32)
            nc.sync.dma_start(out=cb_sb, in_=d_cb.ap())
            w2c_sb = consts.tile([H, C], BF16)
            nc.sync.dma_start(out=w2c_sb, in_=d_w2c.ap())
            ner_col = consts.tile([NLOC, 1], F32)
            nc.scalar.dma_start(out=ner_col, in_=d_ner.ap())
            nf_b = consts.tile([NLOC, BS], F32)
            nc.scalar.dma_start(out=nf_b, in_=d_nfb.ap())
            px_b = consts.tile([NLOC, BS], F32)
            nc.scalar.dma_start(out=px_b, in_=d_pxb.ap())
            if with_c0:
                ones_row_bf = consts.tile([1, 128], BF16)
                nc.vector.memset(ones_row_bf, 1.0)
                c0_sb = consts.tile([1, C], BF16)
                nc.sync.dma_start(out=c0_sb, in_=d_c0.ap())

            # ---- head: X1 = relu(W1h.T @ [embT | all_fT] + b1h) ----
            xt = phead.tile([128, 192], F32, tag="ph")
            for i in range(DCH):
                nc.tensor.matmul(
                    xt, w1h_sb[:, i, :], etf[:, i, :], start=(i == 0),
                    stop=(i == DCH - 1),
                )
            x1 = work.tile([128, 192], BF16)
            nc.scalar.activation(x1, xt, ACTF.Relu, bias=b1h_sb)
            # layer 2 (no bias: head_b2 folded into ca/cb)
            x2p = phead.tile([128, 192], F32, tag="ph")
            nc.tensor.matmul(x2p, wh2_sb, x1)
            x2 = work.tile([128, 192], BF16)
            nc.vector.tensor_copy(x2, x2p)
            # alphaT [H', bs], betaT [H', nloc] with constants folded in
            abp = phead.tile([128, 192], F32, tag="ph")
            nc.tensor.matmul(abp[:, 0:128], bm_sb, x2[:, 0:128])
            nc.tensor.matmul(abp[:, 128:192], am_sb, x2[:, 128:192])
            beta_bf = work.tile([128, NLOC], BF16)
            nc.vector.tensor_scalar_add(beta_bf, abp[:, 0:128], cb_sb)
            alpha_bf = work.tile([128, BS], BF16)
            nc.vector.tensor_scalar_add(alpha_bf, abp[:, 128:192], ca_sb)

            # ---- z_x scores (transposed): ezT[n, b] = exp(cos/(T*sqrt(N))) ----
            slp = psmall.tile([NLOC, BS], F32, tag="ps")
            for i in range(DCH):
                nc.tensor.matmul(
                    slp, etf[:, i, 0:128], etf[:, i, 128:192], start=(i == 0),
                    stop=(i == DCH - 1),
                )
            t3 = work.tile([NLOC, BS], F32)
            nc.vector.tensor_tensor(t3, slp, nf_b, op=ALU.mult)
            ez = work.tile([NLOC, BS], F32)
            nc.scalar.activation(ez, t3, ACTF.Exp, scale=ner_col)

            # ---- h1T quarters: u_k[:, j, :] = relu(betaT + alphaT[:, 16k+j]) ----
            u_tiles = []
            for k in range(UCH):
                uk = bigp.tile([128, UB, NLOC], BF16, tag=f"u{k}")
                nc.vector.tensor_tensor(
                    uk,
                    beta_bf.unsqueeze(1).broadcast_to([128, UB, NLOC]),
                    alpha_bf[:, UB * k : UB * (k + 1)]
                    .unsqueeze(2)
                    .broadcast_to([128, UB, NLOC]),
                    op=ALU.add,
                )
                if k % 2 == 0:
                    nc.scalar.activation(uk, uk, ACTF.Relu)
                else:
                    nc.vector.tensor_scalar_max(uk, uk, 0.0)
                u_tiles.append(uk)

            # ---- main loop: logits (4 b per psum bank, 8 per 2-bank tile) ----
            e_t = bigp.tile([128, BS * C], F32)       # exp(logits), b-major packed
            s_t = work.tile([128, BS], F32)           # softmax denominators
            w_t = work.tile([128, BS], F32)           # p_x[b] / S[n, b]
            e2q_tiles = []
            for q in range(UCH):
                e2q = bigp.tile([128, C, UB], F32, tag=f"e2{q}")
                e2q_tiles.append(e2q)
            sxp = work.tile([128, UCH, C], F32)       # per-quarter partial sums
            for g in range(SG):
                pg = pbig.tile([128, 1024], F32, tag="pb")
                for j in range(GRP):
                    b = GRP * g + j
                    h1t = u_tiles[b // UB][:, b % UB, :]
                    off = 512 * (j // 4) + C * (j % 4)
                    sl = pg[:, off : off + C]
                    if with_c0:
                        nc.tensor.matmul(
                            sl, ones_row_bf, c0_sb, start=True, stop=False
                        )
                        nc.tensor.matmul(sl, h1t, w2c_sb, start=False, stop=True)
                    else:
                        nc.tensor.matmul(sl, h1t, w2c_sb, start=True, stop=True)
                pg_v = pg.rearrange("p (u x) -> p u x", u=2)[:, :, 0 : 4 * C]
                pg_v = pg_v.rearrange("p u (j c) -> p u j c", c=C)
                egs = e_t[:, GRP * C * g : GRP * C * (g + 1)]
                nc.scalar.activation(
                    egs.rearrange("p (u j c) -> p u j c", u=2, j=4), pg_v, ACTF.Exp
                )
                if g % 2 == 1:
                    q = g // 2
                    qsl = slice(UB * q, UB * (q + 1))
                    e_v = e_t[:, UB * C * q : UB * C * (q + 1)].rearrange(
                        "p (b c) -> p b c", c=C
                    )
                    nc.vector.reduce_sum(s_t[:, qsl], e_v, axis=AX.X)
                    nc.vector.reciprocal(w_t[:, qsl], s_t[:, qsl])
                    nc.vector.tensor_tensor(
                        w_t[:, qsl], w_t[:, qsl], px_b[:, qsl], op=ALU.mult
                    )
                    w_v = w_t[:, qsl].unsqueeze(2).broadcast_to([128, UB, C])
                    nc.gpsimd.tensor_tensor(
                        e2q_tiles[q].transpose([0, 2, 1]), e_v, w_v, op=ALU.mult
                    )
                    nc.vector.reduce_sum(sxp[:, q, :], e2q_tiles[q], axis=AX.X)

            # ---- sum the quarter partials, then out = ezT.T @ [sum_x | 1] ----
            sxo = work.tile([128, C + 1], F32)
            nc.vector.memset(sxo[:, C : C + 1], 1.0)
            sxp_v = bass.AP(
                tensor=sxp.tensor,
                offset=sxp.offset,
                ap=[list(sxp.ap[0]), [1, C], [C, UCH]],
            )
            nc.vector.reduce_sum(sxo[:, 0:C], sxp_v, axis=AX.X)
            onp = psmall.tile([BS, C + 1], F32, tag="ps")
            nc.tensor.matmul(onp, ez, sxo)
            on_s = work.tile([BS, C + 1], F32)
            nc.vector.tensor_copy(on_s, onp)
            nc.sync.dma_start(out=d_out.ap(), in_=on_s)

    nc.compile()
    return nc


_CACHE: dict = {}
LAST_RESULTS = None  # BassKernelResults of the most recent run (for profiling)


def _get_nc(with_c0: bool):
    if with_c0 not in _CACHE:
        _CACHE[with_c0] = _build(with_c0)
    return _CACHE[with_c0]


def kernel(
    all_f, embedding, all_y,
    head_w1, head_b1, head_g, head_beta, head_rm, head_rv, head_w2, head_b2,
    mix_w1, mix_b1, mix_g, mix_beta, mix_rm, mix_rv, mix_w2, mix_b2,
    clf_w, clf_b,
):
    f64 = np.float64
    bf16 = ml_dtypes.bfloat16
    sh = head_g.astype(f64) / np.sqrt(head_rv.astype(f64) + BN_EPS)
    th = head_beta.astype(f64) - head_rm.astype(f64) * sh
    w1h = (head_w1.astype(f64) * sh[None, :]).astype(bf16)
    b1h = (head_b1.astype(f64) * sh + th).astype(np.float32)[:, None]
    sm = mix_g.astype(f64) / np.sqrt(mix_rv.astype(f64) + BN_EPS)
    tm = mix_beta.astype(f64) - mix_rm.astype(f64) * sm
    am = mix_w1[:H].astype(f64) * sm[None, :]
    bm = mix_w1[H:].astype(f64) * sm[None, :]
    cm = mix_b1.astype(f64) * sm + tm
    ca = (head_b2.astype(f64) @ am + cm).astype(np.float32)[:, None]
    cb = (head_b2.astype(f64) @ bm).astype(np.float32)[:, None]
    w2c = (mix_w2.astype(f64) @ clf_w.astype(f64)).astype(bf16)
    c0 = (mix_b2.astype(f64) @ clf_w.astype(f64) + clf_b.astype(f64)).astype(
        np.float32
    )
    with_c0 = bool(np.any(c0 != 0.0))

    af = np.ascontiguousarray(all_f, dtype=np.float32)
    emb = np.ascontiguousarray(embedding, dtype=np.float32)
    # input-side host prep: transposed layouts, row norms, p_x
    nf = 1.0 / np.sqrt((af.astype(f64) ** 2).sum(axis=1))          # [bs]
    nf_b = np.broadcast_to(nf[None, :], (NLOC, BS)).astype(np.float32)
    gscore = ((af @ af.T).astype(f64) * nf[:, None] * nf[None, :]).sum(axis=1)
    gscore = gscore / (T * np.sqrt(BS))
    pe = np.exp(gscore - gscore.max())
    px = pe / pe.sum()                                              # [bs]
    px_b = np.broadcast_to(px[None, :], (NLOC, BS)).astype(np.float32)
    aft = np.ascontiguousarray(af.T).astype(bf16)                   # [D, bs]

    base = {
        "w1h": w1h,
        "b1h": b1h,
        "wh2": np.ascontiguousarray(head_w2).astype(bf16),
        "am": am.astype(bf16),
        "bm": bm.astype(bf16),
        "ca": ca,
        "cb": cb,
        "w2c": w2c,
        "nfb": np.ascontiguousarray(nf_b),
        "pxb": np.ascontiguousarray(px_b),
    }
    if with_c0:
        base["c0t"] = c0[None, :].astype(bf16)

    in_maps = []
    for i in range(NCORES):
        shard = emb[i * NLOC : (i + 1) * NLOC]
        etf = np.empty((D, 192), dtype=bf16)
        etf[:, 0:128] = shard.T.astype(bf16)
        etf[:, 128:192] = aft
        ne = 1.0 / np.sqrt((shard.astype(f64) ** 2).sum(axis=1))
        ner = (ne / (T * np.sqrt(N))).astype(np.float32)[:, None]
        in_maps.append(dict(base, etf=etf, ner=ner))

    nc = _get_nc(with_c0)
    res = bass_utils.run_bass_kernel_spmd(nc, in_maps, core_ids=list(range(NCORES)))
    global LAST_RESULTS
    LAST_RESULTS = res
    parts = np.stack([r["out_nd"] for r in res.results], axis=0)  # [8, 64, 66]
    tot = parts.sum(axis=0)
    return (tot[:, :C] / tot[:, C : C + 1]).astype(np.float32)
